# revision 25
# baseline (speedup 1.0000x reference)
"""Trainium2 Bass kernel for a dense transformer block (LN-attn-LN-MLP).

Sharding: core pair (2b, 2b+1) handles batch b. Each core computes 8 of the
16 attention heads over the full 2048-token sequence (head/tensor parallel).
Attention is pipelined by query chunk; the per-chunk c_proj partials are
ReduceScattered (bf16) across the pair, both collectives hidden behind
remaining attention / fc1 compute. The MLP runs on each core's 1024-token
half.

All activations stay feature-major [features, tokens]. LayerNorm gamma/beta
and all matmul biases are folded on the host; LN stats are computed with an
all-ones [128,128] lhsT so the per-token sum/sumsq arrive in PSUM already
broadcast across all partitions (no cross-partition moves at all), and
centering (x-mu)*rstd runs on the Vector engine, so no tail k-tiles are
needed anywhere. Softmax: scores computed transposed [k, q]; exp on ScalarE;
causal masking via a DVE multiply with a constant triangular tile; the row
sum comes from a ones-column PREPENDED to V (lands on PSUM partition 0) and
is broadcast across partitions with a rank-1 outer-product matmul. The
gpsimd engine is reserved for the collectives (it stalls during them).
Weights are pre-tiled on the host so every weight DMA is contiguous per
partition.
"""

import os
import sys

sys.path.insert(0, "/opt/trn_rl_repo")

import ml_dtypes
import numpy as np

import concourse.bass as bass
import concourse.tile as tile
from concourse import bacc, mybir
from concourse.bass_utils import run_bass_kernel_spmd

F32 = mybir.dt.float32
F32R = mybir.dt.float32r
BF16 = mybir.dt.bfloat16
AF = mybir.ActivationFunctionType
ALU = mybir.AluOpType

# Model dims
C = 1024            # embed
T = 2048            # sequence length (full context per core)
B = 4               # batch
D = 64              # head dim
HL = 8              # local heads per core
TO = 1024           # output tokens per core (after pair ReduceScatter)
CH = 512            # token chunk (matmul free dim)
KB = 128            # key block
FF = 4 * C          # 4096
EPS = 1e-5
KT = C // 128       # 8 k-tiles over embed dim
KT_P = HL * D // 128   # 4 proj k-tiles
KT_F = FF // 128       # 32

# bias column layout in the packed [128, 56] bias tensor
BQ, BK, BP, BF1, BF2 = 0, 4, 8, 16, 48


def _r(ap):
    return ap.bitcast(F32R) if ap.dtype == F32 else ap


def _emit(tc, io):
    nc = tc.nc
    xT, xres, wq, wk, wv, wp, wfc, wfc2, bias_d, out_ap = (
        io["xT"], io["xres"], io["wq"], io["wk"], io["wv"], io["wp"],
        io["wfc"], io["wfc2"], io["biases"], io["out"])

    const = tc.alloc_tile_pool(name="const", bufs=1)
    ones_s = const.tile([128, 128], F32)
    nc.vector.memset(ones_s[:], 1.0)
    o128 = const.tile([1, 128], F32R)          # rank-1 broadcast lhsT (LN)
    nc.vector.tensor_copy(o128[:], ones_s[0:1, :])
    onec = const.tile([128, 1], F32R)          # ones column (LN1 stats)
    nc.vector.tensor_copy(onec[:], ones_s[:, 0:1])
    onec_bf = const.tile([128, 1], BF16)       # ones column (LN2 stats)
    nc.vector.tensor_copy(onec_bf[:], ones_s[:, 0:1])
    btile = const.tile([128, 56], F32)
    nc.sync.dma_start(btile[:], bias_d)
    eps128 = const.tile([128, 1], F32)
    nc.vector.memset(eps128[:], EPS)
    # causal triangular mask for the diagonal 128x128 blocks (DVE multiply)
    tri = const.tile([128, KB], BF16)
    nc.vector.memset(tri[:], 1.0)
    nc.gpsimd.affine_select(tri[:], tri[:], pattern=[[1, KB]],
                            compare_op=ALU.is_ge, fill=0.0, base=0,
                            channel_multiplier=-1)

    # ---------------- pools (stack / LIFO: long-lived deepest) ----------------
    sq = tc.alloc_tile_pool(name="sq", bufs=1)
    st = tc.alloc_tile_pool(name="st", bufs=1)
    xrp = tc.alloc_tile_pool(name="xrp", bufs=1)
    # left, attention-era:
    wqkP = tc.alloc_tile_pool(name="wqkP", bufs=1)
    wvP = tc.alloc_tile_pool(name="wvP", bufs=1)
    wpP = tc.alloc_tile_pool(name="wpP", bufs=1)
    xin = tc.alloc_tile_pool(name="xin", bufs=1)
    qTp = tc.alloc_tile_pool(name="qTp", bufs=1)
    etp = tc.alloc_tile_pool(name="etp", bufs=1)
    ynp = tc.alloc_tile_pool(name="ynp", bufs=1)
    yTp = tc.alloc_tile_pool(name="yTp", bufs=1)
    prj = tc.alloc_tile_pool(name="prj", bufs=1)
    # right side:
    x2P = tc.alloc_tile_pool(name="x2P", bufs=1, side="right")
    xs2P = tc.alloc_tile_pool(name="xs2P", bufs=1, side="right")
    xsP = tc.alloc_tile_pool(name="xsP", bufs=1, side="right")
    kTP = tc.alloc_tile_pool(name="kTP", bufs=1, side="right")
    vSP = tc.alloc_tile_pool(name="vSP", bufs=1, side="right")
    # PSUM
    mm_ps = tc.alloc_tile_pool(name="mm_ps", bufs=1, space="PSUM")
    sc_ps = tc.alloc_tile_pool(name="sc_ps", bufs=1, space="PSUM")
    av_ps = tc.alloc_tile_pool(name="av_ps", bufs=1, space="PSUM")
    dram = tc.alloc_tile_pool(name="dram", bufs=1, space="DRAM")

    xs = {}
    kT = [kTP.tile([128, T], BF16, tag=f"kT{hp}", name=f"kT{hp}")
          for hp in range(4)]
    v_sb = [vSP.tile([128, HL, D + 1], BF16, tag=f"v{tb}", name=f"v{tb}")
            for tb in range(T // 128)]
    for tb in range(T // 128):
        nc.vector.memset(v_sb[tb][:, :, 0:1], 1.0)
    x2 = [x2P.tile([128, TO], BF16, tag=f"x2_{k}", name=f"x2_{k}")
          for k in range(KT)]
    xs2 = [xs2P.tile([128, TO], BF16, tag=f"xs2_{k}", name=f"xs2_{k}")
           for k in range(KT)]

    # weights are host-pre-tiled: every weight DMA is contiguous per partition
    wq_t = [wqkP.tile([128, KT, 128], BF16, tag=f"wq{hp}", name=f"wq{hp}")
            for hp in range(4)]
    wk_t = [wqkP.tile([128, KT, 128], BF16, tag=f"wk{hp}", name=f"wk{hp}")
            for hp in range(4)]
    for hp in range(4):
        nc.sync.dma_start(wq_t[hp][:], wq[:, hp * C:(hp + 1) * C])
        nc.sync.dma_start(wk_t[hp][:], wk[:, hp * C:(hp + 1) * C])
    wv_t = wvP.tile([128, KT, HL * D], BF16, tag="wv")
    nc.sync.dma_start(wv_t[:], wv)
    wp_t = wpP.tile([128, KT_P, C], BF16, tag="wp")
    nc.sync.dma_start(wp_t[:], wp)

    rs_in = [dram.tile([2, C, CH], BF16, tag=f"rsin{i}", name=f"rsin{i}")
             for i in range(2)]
    rs_out = [dram.tile([C, CH], BF16, tag=f"rsout{i}", name=f"rsout{i}")
              for i in range(2)]

    def _ln_core(pref, sum_ps, sq_ps, get_x, put_xs, n_k, ps_pool, ps_tag,
                 ps_bufs):
        """LN tail: row stats -> outer-product broadcast -> centered out."""
        srow = st.tile([1, 2 * CH], F32R, tag="srow", bufs=1, name=f"{pref}sr")
        nc.scalar.copy(srow[0:1, 0:CH], sum_ps[:])
        nc.scalar.copy(srow[0:1, CH:], sq_ps[:])
        sum_b = ps_pool.tile([128, CH], F32, tag=ps_tag, bufs=ps_bufs,
                             name=f"{pref}sbb")
        nc.tensor.matmul(sum_b[:], o128[:], srow[0:1, 0:CH],
                         start=True, stop=True)
        sq_b = ps_pool.tile([128, CH], F32, tag=ps_tag, bufs=ps_bufs,
                            name=f"{pref}qbb")
        nc.tensor.matmul(sq_b[:], o128[:], srow[0:1, CH:],
                         start=True, stop=True)
        mu_b = st.tile([128, CH], F32, tag="mub", bufs=2, name=f"{pref}mu")
        nc.vector.tensor_scalar_mul(mu_b[:], sum_b[:], 1.0 / C)
        var = st.tile([128, CH], F32, tag="var", bufs=2, name=f"{pref}var")
        nc.vector.tensor_scalar_mul(var[:], sq_b[:], 1.0 / C)
        mu2 = sq.tile([128, CH], F32, tag="ct", bufs=2, name=f"{pref}mu2")
        nc.vector.tensor_mul(mu2[:], mu_b[:], mu_b[:])
        nc.vector.tensor_tensor(out=var[:], in0=var[:], in1=mu2[:],
                                op=ALU.subtract)
        nc.scalar.activation(var[:], var[:], AF.Sqrt, bias=eps128[:])
        nc.vector.reciprocal(var[:], var[:])
        for k in range(n_k):
            ct = sq.tile([128, CH], F32, tag="ct", bufs=2, name=f"{pref}ct{k}")
            nc.vector.tensor_tensor(out=ct[:], in0=get_x(k), in1=mu_b[:],
                                    op=ALU.subtract)
            nc.vector.tensor_mul(put_xs(k), ct[:], var[:])

    def preproc(c):
        """LN1 for chunk c -> xs bf16 (centered, scaled)."""
        csl = slice(c * CH, (c + 1) * CH)
        xt = []
        sum_ps = mm_ps.tile([1, CH], F32, tag="g", bufs=2, name=f"sum{c}")
        sq_ps = mm_ps.tile([1, CH], F32, tag="g", bufs=2, name=f"sq{c}")
        for k in range(KT):
            t = xin.tile([128, CH], F32R, tag="x", bufs=10, name=f"xin{c}_{k}")
            nc.gpsimd.dma_start(t[:], xT[k * 128:(k + 1) * 128, csl])
            xt.append(t)
            x2q = sq.tile([128, CH], F32R, tag="sq", bufs=2, name=f"x2q{c}_{k}")
            nc.vector.tensor_mul(x2q[:], t[:], t[:])
            nc.tensor.matmul(sum_ps[:], onec[:], t[:],
                             start=(k == 0), stop=(k == KT - 1))
            nc.tensor.matmul(sq_ps[:], onec[:], x2q[:],
                             start=(k == 0), stop=(k == KT - 1))
        def put_xs(k):
            xk = xsP.tile([128, CH], BF16, tag=f"xs{k}", bufs=2,
                          name=f"xs{c}_{k}")
            xs[(c, k)] = xk
            return xk[:]
        _ln_core(f"p{c}", sum_ps, sq_ps, lambda k: xt[k][:], put_xs, KT,
                 mm_ps, "g", 2)

    def kv(c):
        csl = slice(c * CH, (c + 1) * CH)
        for hp in range(4):
            kp = mm_ps.tile([128, CH], F32, tag="g", bufs=2, name=f"kps{c}_{hp}")
            for k in range(KT):
                nc.tensor.matmul(kp[:], wk_t[hp][:, k, :], xs[(c, k)][:],
                                 start=(k == 0), stop=(k == KT - 1))
            nc.vector.tensor_scalar_add(kT[hp][:, csl], kp[:],
                                        btile[:, BK + hp:BK + hp + 1])
        for tb in range(c * CH // 128, (c + 1) * CH // 128):
            tsl = slice(tb * 128 - c * CH, tb * 128 - c * CH + 128)
            vp = mm_ps.tile([128, HL * D], F32, tag="g", bufs=2, name=f"vps{tb}")
            for k in range(KT):
                nc.tensor.matmul(vp[:], xs[(c, k)][:, tsl], wv_t[:, k, :],
                                 start=(k == 0), stop=(k == KT - 1))
            nc.vector.tensor_copy(
                v_sb[tb][:, :, 1:D + 1],
                vp[:].rearrange("p (h d) -> p h d", h=HL))

    def q_chunk(c):
        qts = []
        for hp in range(4):
            qp = mm_ps.tile([128, CH], F32, tag="g", bufs=2, name=f"qps{c}_{hp}")
            for k in range(KT):
                nc.tensor.matmul(qp[:], wq_t[hp][:, k, :], xs[(c, k)][:],
                                 start=(k == 0), stop=(k == KT - 1))
            qt = qTp.tile([128, CH], BF16, tag=f"q{hp}", bufs=2,
                          name=f"qT{c}_{hp}")
            nc.vector.tensor_scalar_add(qt[:], qp[:],
                                        btile[:, BQ + hp:BQ + hp + 1])
            qts.append(qt)
        return qts

    def attend(c, qts):
        yt = [yTp.tile([128, CH], BF16, tag=f"y{k}", bufs=3,
                       name=f"yT{c}_{k}") for k in range(KT_P)]
        n_kb = (c + 1) * (CH // KB)
        for hp in range(4):
            avs = []
            for hh in range(2):
                avs.append(av_ps.tile([D + 1, CH], F32, tag=f"a{hh}", bufs=1,
                                      name=f"av{c}_{hp}_{hh}"))
            for kb in range(n_kb):
                diag_j = kb - (n_kb - CH // KB)
                q0 = max(0, diag_j * KB)
                ksl = slice(kb * KB, (kb + 1) * KB)
                sp = sc_ps.tile([128, 2, CH], F32, tag="s", bufs=2,
                                name=f"sc{c}_{hp}_{kb}")
                for hh in range(2):
                    prow = slice(hh * D, (hh + 1) * D)
                    nc.tensor.matmul(sp[:, hh, q0:], kT[hp][prow, ksl],
                                     qts[hp][prow, q0:], start=True, stop=True)
                et = etp.tile([128, 2, CH], BF16, tag="e", bufs=2,
                              name=f"et{c}_{hp}_{kb}")
                nc.scalar.activation(et[:, :, q0:], sp[:, :, q0:], AF.Exp,
                                     scale=1.0 / np.sqrt(D))
                if diag_j >= 0:
                    for hh in range(2):
                        nc.vector.tensor_mul(et[:, hh, q0:q0 + KB],
                                             et[:, hh, q0:q0 + KB], tri[:])
                for hh in range(2):
                    nc.tensor.matmul(avs[hh][:, q0:],
                                     v_sb[kb][:, hp * 2 + hh, :],
                                     et[:, hh, q0:],
                                     start=(kb == 0), stop=(kb == n_kb - 1))
            for hh in range(2):
                h_loc = hp * 2 + hh
                # copy out fast (frees the av psum slot), then broadcast the
                # sum row via a tiny DRAM bounce -- everything off the PE
                yu = ynp.tile([D + 1, CH], BF16, tag="yu", bufs=2,
                              name=f"yu{c}_{h_loc}")
                nc.vector.tensor_copy(yu[:], avs[hh][:, :])
                row = dram.tile([1, CH], BF16, tag="nrow", bufs=2,
                                name=f"nr{c}_{h_loc}")
                eng = nc.gpsimd if c < 3 else nc.sync
                eng.dma_start(row[:], yu[0:1, :])
                rap = bass.AP(row.tensor, row.offset, [[0, D + 1], [1, CH]])
                sbb = ynp.tile([D + 1, CH], BF16, tag="sbb", bufs=2,
                               name=f"sbb{c}_{h_loc}")
                eng.dma_start(sbb[:], rap)
                sb = ynp.tile([D + 1, CH], F32, tag="sb", bufs=2,
                              name=f"sb{c}_{h_loc}")
                nc.vector.reciprocal(sb[:], sbb[:])
                yn = ynp.tile([D + 1, CH], BF16, tag="yn", bufs=2,
                              name=f"yn{c}_{h_loc}")
                nc.vector.tensor_mul(yn[:], yu[:], sb[:])
                nc.sync.dma_start(
                    yt[h_loc // 2][(h_loc % 2) * D:(h_loc % 2 + 1) * D, :],
                    yn[1:D + 1, :])
        return yt

    def proj(c, yt, rs_idx, slot):
        for ob in range(C // 128):
            osl = slice(ob * 128, (ob + 1) * 128)
            pp = mm_ps.tile([128, CH], F32, tag="g", bufs=2, name=f"pr{c}_{ob}")
            for k in range(KT_P):
                nc.tensor.matmul(pp[:], wp_t[:, k, osl], yt[k][:],
                                 start=(k == 0), stop=(k == KT_P - 1))
            pt = prj.tile([128, CH], BF16, tag="p", bufs=2, name=f"prs{c}_{ob}")
            nc.vector.tensor_scalar_add(pt[:], pp[:],
                                        btile[:, BP + ob:BP + ob + 1])
            nc.gpsimd.dma_start(rs_in[rs_idx][slot, osl, :], pt[:])

    def rs_go(i):
        nc.gpsimd.collective_compute(
            "ReduceScatter", ALU.add,
            replica_groups=[[0, 1], [2, 3], [4, 5], [6, 7]],
            ins=[rs_in[i].opt()], outs=[rs_out[i].opt()])

    def ln2res(h, ps_pool, ps_tag, ps_bufs):
        hsl = slice(h * CH, (h + 1) * CH)
        for k in range(KT):
            rr = xrp.tile([128, CH], BF16, tag="rr", bufs=3, name=f"rr{h}_{k}")
            nc.gpsimd.dma_start(rr[:], rs_out[h][k * 128:(k + 1) * 128, :])
            xr = xrp.tile([128, CH], F32, tag="xr", bufs=3, name=f"xr{h}_{k}")
            nc.gpsimd.dma_start(xr[:], xres[k * 128:(k + 1) * 128, hsl])
            nc.vector.tensor_add(x2[k][:, hsl], rr[:], xr[:])
        sum_ps = ps_pool.tile([1, CH], F32, tag=ps_tag, bufs=ps_bufs,
                              name=f"l2sum{h}")
        sq_ps = ps_pool.tile([1, CH], F32, tag=ps_tag, bufs=ps_bufs,
                             name=f"l2sq{h}")
        for k in range(KT):
            x2q = sq.tile([128, CH], BF16, tag="sq2", bufs=2, name=f"l2q{h}_{k}")
            nc.vector.tensor_mul(x2q[:], x2[k][:, hsl], x2[k][:, hsl])
            nc.tensor.matmul(sum_ps[:], onec_bf[:], x2[k][:, hsl],
                             start=(k == 0), stop=(k == KT - 1))
            nc.tensor.matmul(sq_ps[:], onec_bf[:], x2q[:],
                             start=(k == 0), stop=(k == KT - 1))
        _ln_core(f"l{h}", sum_ps, sq_ps,
                 lambda k: x2[k][:, hsl], lambda k: xs2[k][:, hsl], KT,
                 ps_pool, ps_tag, ps_bufs)

    # ---------------- phase A: LN1 + attention, chunk-pipelined ----------------
    preproc(0)
    kv(0)
    q0 = q_chunk(0)
    preproc(1)
    y0 = attend(0, q0)
    kv(1)
    q1 = q_chunk(1)
    preproc(2)
    y1 = attend(1, q1)
    kv(2)
    q2 = q_chunk(2)
    preproc(3)
    y2 = attend(2, q2)
    kv(3)
    q3 = q_chunk(3)
    proj(0, y0, 0, 0)
    proj(1, y1, 1, 0)
    proj(2, y2, 0, 1)
    rs_go(0)
    y3 = attend(3, q3)
    proj(3, y3, 1, 1)
    rs_go(1)
    ln2res(0, mm_ps, "g", 2)

    for p in (prj, yTp, ynp, etp, qTp, xin, wpP, wvP, wqkP,
              vSP, kTP, xsP,
              av_ps, sc_ps):
        p.release()

    # ---------------- phase B: MLP ----------------
    hTP = tc.alloc_tile_pool(name="hTP", bufs=1, side="right")
    fcw = tc.alloc_tile_pool(name="fcw", bufs=1)
    fc_ps = tc.alloc_tile_pool(name="fc_ps", bufs=1, space="PSUM")
    fc2w = tc.alloc_tile_pool(name="fc2w", bufs=1)
    fc2_ps = tc.alloc_tile_pool(name="fc2_ps", bufs=1, space="PSUM")
    out_sb = tc.alloc_tile_pool(name="out_sb", bufs=1)

    hT = [[None] * KT_F for _ in range(2)]

    def fc1(h):
        hsl = slice(h * CH, (h + 1) * CH)
        for f in range(KT_F):
            wt = fcw.tile([128, KT, 128], BF16, tag="w", bufs=6,
                          name=f"wfc{h}_{f}")
            nc.sync.dma_start(wt[:], wfc[:, f * C:(f + 1) * C])
            hp_ps = fc_ps.tile([128, CH], F32, tag="f", bufs=3,
                               name=f"fc1p{h}_{f}")
            for k in range(KT):
                nc.tensor.matmul(hp_ps[:], wt[:, k, :], xs2[k][:, hsl],
                                 start=(k == 0), stop=(k == KT - 1))
            ht = hTP.tile([128, CH], BF16, tag=f"h{h}_{f}", name=f"hT{h}_{f}")
            nc.scalar.activation(ht[:], hp_ps[:], AF.Gelu_apprx_tanh,
                                 bias=btile[:, BF1 + f:BF1 + f + 1])
            hT[h][f] = ht

    fc1(0)
    ln2res(1, fc_ps, "f", 3)
    fc1(1)

    for ob in range(C // 128):
        osl = slice(ob * 128, (ob + 1) * 128)
        w2 = fc2w.tile([128, KT_F, 128], BF16, tag="w2", bufs=2,
                       name=f"w2_{ob}")
        nc.sync.dma_start(w2[:], wfc2[:, ob * FF:(ob + 1) * FF])
        for h in range(2):
            hsl = slice(h * CH, (h + 1) * CH)
            op_ps = fc2_ps.tile([128, CH], F32, tag="o", bufs=2,
                                name=f"fc2p{ob}_{h}")
            for j in range(KT_F):
                nc.tensor.matmul(op_ps[:], w2[:, j, :], hT[h][j][:],
                                 start=(j == 0), stop=(j == KT_F - 1))
            ot = out_sb.tile([128, CH], F32, tag="ot", bufs=3,
                             name=f"ot{ob}_{h}")
            nc.vector.scalar_tensor_tensor(
                out=ot[:], in0=op_ps[:], scalar=btile[:, BF2 + ob:BF2 + ob + 1],
                in1=x2[ob][:, hsl], op0=ALU.add, op1=ALU.add)
            nc.sync.dma_start(out_ap[osl, hsl], ot[:])

    for p in (out_sb, fc2w, fcw,
              xrp, st, sq,
              hTP, xs2P, x2P,
              fc2_ps, fc_ps, mm_ps,
              dram, const):
        p.release()


_NC_CACHE = None


def _build():
    global _NC_CACHE
    if _NC_CACHE is not None:
        return _NC_CACHE
    nc = bacc.Bacc("TRN2", target_bir_lowering=False, debug=False, num_devices=8)
    io = {
        "xT": nc.dram_tensor("xT", [C, T], F32R, kind="ExternalInput").ap(),
        "xres": nc.dram_tensor("xres", [C, TO], F32, kind="ExternalInput").ap(),
        "wq": nc.dram_tensor("wq", [128, 4 * C], BF16, kind="ExternalInput").ap(),
        "wk": nc.dram_tensor("wk", [128, 4 * C], BF16, kind="ExternalInput").ap(),
        "wv": nc.dram_tensor("wv", [128, KT, HL * D], BF16,
                             kind="ExternalInput").ap(),
        "wp": nc.dram_tensor("wp", [128, KT_P, C], BF16,
                             kind="ExternalInput").ap(),
        "wfc": nc.dram_tensor("wfc", [128, KT_F * C], BF16,
                              kind="ExternalInput").ap(),
        "wfc2": nc.dram_tensor("wfc2", [128, 8 * FF], BF16,
                               kind="ExternalInput").ap(),
        "biases": nc.dram_tensor("biases", [128, 56], F32,
                                 kind="ExternalInput").ap(),
        "out": nc.dram_tensor("out", [C, TO], F32, kind="ExternalOutput").ap(),
    }
    with tile.TileContext(nc) as tc:
        _emit(tc, io)
    nc.compile()
    _NC_CACHE = nc
    return nc


def _tile_w_blocks(w, kt, nsub):
    """[kt*128, nsub*Fs] -> [128, nsub*kt*Fs] with per-partition layout
    [n][k][Fs]: slice [:, n*kt*Fs:(n+1)*kt*Fs] viewed as [kt, Fs] equals
    w[k*128+p, n*Fs + f]."""
    rows, cols = w.shape
    assert rows == kt * 128 and cols % nsub == 0
    fs = cols // nsub
    return np.ascontiguousarray(
        w.reshape(kt, 128, nsub, fs).transpose(1, 2, 0, 3).reshape(128, -1))


def kernel(x, ln1_g, ln1_b, w_attn, b_attn, w_proj, b_proj,
           ln2_g, ln2_b, w_fc, b_fc, w_fc2, b_fc2):
    f32 = lambda a: np.asarray(a, np.float32)
    x = f32(x)
    ln1_g, ln1_b, w_attn, b_attn = f32(ln1_g), f32(ln1_b), f32(w_attn), f32(b_attn)
    w_proj, b_proj, ln2_g, ln2_b = f32(w_proj), f32(b_proj), f32(ln2_g), f32(ln2_b)
    w_fc, b_fc, w_fc2, b_fc2 = f32(w_fc), f32(b_fc), f32(w_fc2), f32(b_fc2)

    nc = _build()
    bf = ml_dtypes.bfloat16

    wq_full = w_attn[:, :C] * ln1_g[:, None]
    wk_full = w_attn[:, C:2 * C] * ln1_g[:, None]
    wv_full = w_attn[:, 2 * C:] * ln1_g[:, None]
    bq_full = ln1_b @ w_attn[:, :C] + b_attn[:C]
    bk_full = ln1_b @ w_attn[:, C:2 * C] + b_attn[C:2 * C]
    bv_full = ln1_b @ w_attn[:, 2 * C:] + b_attn[2 * C:]
    wfc_fold = w_fc * ln2_g[:, None]
    bfc1 = ln2_b @ w_fc + b_fc
    wfc_t = _tile_w_blocks(wfc_fold, KT, KT_F).astype(bf)   # [128, 32*C]
    wfc2_t = _tile_w_blocks(w_fc2, KT_F, 8).astype(bf)      # [128, 8*FF]

    in_maps = []
    for core in range(8):
        b_idx, r = core // 2, core % 2
        hsl = slice(r * HL * D, (r + 1) * HL * D)
        xT_b = np.ascontiguousarray(x[b_idx].T)                # [C, T]
        wp_loc = w_proj[hsl, :]                                # [512, C]
        bproj_loc = b_proj / 2.0 + bv_full[hsl] @ wp_loc
        biases = np.zeros((128, 56), np.float32)
        biases[:, BQ:BQ + 4] = bq_full[hsl].reshape(4, 128).T
        biases[:, BK:BK + 4] = bk_full[hsl].reshape(4, 128).T
        biases[:, BP:BP + 8] = bproj_loc.reshape(8, 128).T
        biases[:, BF1:BF1 + 32] = bfc1.reshape(32, 128).T
        biases[:, BF2:BF2 + 8] = b_fc2.reshape(8, 128).T
        in_maps.append({
            "xT": xT_b,
            "xres": np.ascontiguousarray(xT_b[:, r * TO:(r + 1) * TO]),
            "wq": _tile_w_blocks(wq_full[:, hsl], KT, 4).astype(bf),
            "wk": _tile_w_blocks(wk_full[:, hsl], KT, 4).astype(bf),
            "wv": _tile_w_blocks(wv_full[:, hsl], KT, 1).astype(bf)
                  .reshape(128, KT, HL * D),
            "wp": _tile_w_blocks(wp_loc, KT_P, 1).astype(bf)
                  .reshape(128, KT_P, C),
            "wfc": wfc_t,
            "wfc2": wfc2_t,
            "biases": biases,
        })

    trace = bool(int(os.environ.get("KERNEL_TRACE", "0")))
    res = run_bass_kernel_spmd(nc, in_maps, core_ids=list(range(8)), trace=trace)
    kernel.last_result = res

    out = np.empty((B, T, C), np.float32)
    for core in range(8):
        b_idx, r = core // 2, core % 2
        out[b_idx, r * TO:(r + 1) * TO, :] = res.results[core]["out"].T
    return out
